# revision 9
# baseline (speedup 1.0000x reference)
"""Dilated attention (LongNet-style) Trainium2 kernel — v4 (fused PV).

Problem: query/key/value (2, 8192, 12, 64) f32. Three dilation groups
(segment lengths 2048/4096/8192, dilation 1/2/4, head slices 0:4/4:8/8:12).
Each group's gather produces independent dense attention over 2048-position
dilated segments; outputs are normalized per (batch, head, channel) by the
sum over all segment positions, and divided by num_groups.

Sharding: 8 cores = 2 batches x 4 "head columns". Core c owns batch c//4 and
heads {j, 4+j, 8+j} where j = c%4 -- exactly 7 dense 2048x2048x64 attention
units per core (4 + 2 + 1 segments), perfectly balanced, with all segments of
any (batch, head) on one core so normalization needs no cross-core traffic.

Precision (validated by numpy simulation of the exact arithmetic): the
x / x.sum normalization amplifies V-path errors ~140x but score/P-path
errors only ~8-15x. So K keeps an fp16 hi/lo pair packed along the
contraction dim (khl rows 0-63 = kh, 64-127 = kl; qhh rows = qh duplicated,
so scores = (kh+kl)^T qh in ONE fp16 matmul), Q and P are single fp16, and
V keeps an fp16 hi/lo pair.

v4 trick: a matmul's cost is its N (moving columns), not M, so the PV pair
+ denominator fuse into ONE matmul by packing the stationary operand as
  lhsT = [vh(ch 0-63) | vl(ch 0-62) | ones] (128 x 128):
output rows 0-63 = p1@vh, rows 64-126 = p1@vl, row 127 = softmax denom.
The HOST adds the hi/lo halves in f64. Channel 63 loses its lo-correction
(+~3e-3 error, channel-diagonal). Per 128x512 unit the PE now does just
2 matmuls (scores + fused PV) = 1296ns/round of 3, making the single ACT
exp pass (1530ns/round) the bottleneck. Sim end-to-end: ~6.6e-3 (thr 2e-2).

Device kernel (same program on all 8 cores, different data):
  - inputs: qhh/khl [128, 14336] fp16 (as above), vhl [128, 14336] fp16
    (the packed 128x128 stationary blocks per (seg, k-block)).
  - per (chunk, k-block) unit (28 q-chunks of 512 x 16 k-blocks):
      S^T = khl_blk.T @ qhh              (PE, 1 MM, PSUM f32)
      p1 = fp16(exp(S^T*0.125/65536 + ln64))   (ACT, PSUM -> SBUF)
      O'[128, 512] += vhl_blk.T @ p1     (PE, 1 MM, f32 PSUM, accum over kb)
  - O' copied PSUM->SBUF (DVE) into a per-segment staging tile, DMA'd to
    DRAM out [128, 14336] f32 once per segment (4 chunks).
Host: num = O'[0:64] (+= O'[64:127] for ch<63), den = O'[127], T = num/den,
then the group normalization (sum over positions) and /3, scattered into
the (2, 8192, 12, 64) output. Positions not in a dilated group stay zero.
"""

import os
import sys

if "/opt/trn_rl_repo" not in sys.path:
    sys.path.insert(0, "/opt/trn_rl_repo")
if "jax" not in sys.modules:
    os.environ.setdefault("JAX_PLATFORMS", "axon")

import numpy as np

import concourse.bass as bass  # noqa: F401
import concourse.mybir as mybir
import concourse.tile as tile
from concourse import bacc
from concourse.bass_utils import run_bass_kernel_spmd

F32 = mybir.dt.float32
F16 = mybir.dt.float16

B, N, H, D = 2, 8192, 12, 64
NSEG = 7           # segments per core
SEG = 2048         # dilated segment length
NCHUNK = NSEG * 4  # 512-wide q chunks per core
NKB = 16           # 128-row k blocks per segment
NUNIT = NCHUNK * NKB
RW = 3             # k-blocks per exp round (3 PSUM banks per ACT span)
QSC = np.float32(256.0)               # fp16 pre-scale for Q/K/V splits
ESC = float(0.125 / (256.0 * 256.0))  # exp scale: 1/sqrt(64) + descale
import math
PBIAS = float(math.log(64.0))         # exp bias: P *= 64, into fp16-normal range

_CACHE = {}
LAST_RESULT = {}


def _build_nc():
    nc = bacc.Bacc("TRN2", target_bir_lowering=False, debug=False,
                   enable_asserts=False, num_devices=8)
    qhh = nc.dram_tensor("qhh", [128, NSEG * SEG], F16, kind="ExternalInput")
    khl = nc.dram_tensor("khl", [128, NSEG * SEG], F16, kind="ExternalInput")
    vhl = nc.dram_tensor("vhl", [128, NSEG * NKB * 128], F16,
                         kind="ExternalInput")
    out = nc.dram_tensor("out", [128, NCHUNK * 512], F32, kind="ExternalOutput")
    qhh_ap, khl_ap, vhl_ap, out_ap = qhh.ap(), khl.ap(), vhl.ap(), out.ap()

    with tile.TileContext(nc) as tc:
        with (
            tc.tile_pool(name="inp", bufs=1) as inp,
            tc.tile_pool(name="pt", bufs=4) as ptp,
            tc.tile_pool(name="osb", bufs=2) as osbp,
            tc.tile_pool(name="score", bufs=2, space="PSUM") as scp,
            tc.tile_pool(name="ot", bufs=2, space="PSUM") as otp,
        ):
            bias_t = inp.tile([128, 1], F32, tag="bias", name="bias_t")
            nc.vector.memset(bias_t[:, :], PBIAS)

            qh_sb, k_sb, v_sb = [], [], []
            for s in range(NSEG):
                qh = inp.tile([128, SEG], F16, tag=f"qh{s}", name=f"qh{s}")
                kk = inp.tile([128, SEG], F16, tag=f"k{s}", name=f"k{s}")
                vv = inp.tile([128, NKB * 128], F16, tag=f"v{s}", name=f"v{s}")
                # split the first segment's transfers across DMA queues so
                # round 0 isn't gated on a single ~512KB queue transfer
                nsl_dma = 4 if s == 0 else 1
                for t, ap_, w in ((qh, qhh_ap, SEG), (kk, khl_ap, SEG),
                                  (vv, vhl_ap, NKB * 128)):
                    step = w // nsl_dma
                    for z in range(nsl_dma):
                        lo = z * step
                        nc.sync.dma_start(
                            t[:, lo:lo + step],
                            ap_[:, s * w + lo:s * w + lo + step])
                qh_sb.append(qh)
                k_sb.append(kk)
                v_sb.append(vv)

            # Warm-up: replay the round-0 scores matmul into a scratch PSUM
            # slot as soon as the seg-0 part-0 DMAs land. Depending on input
            # tiles (not DVE-memset scratch) lets the first matmul fire ~3us
            # earlier than behind the DVE engine preamble, and the dummy exp
            # pulls the ~2.7us ACT table load off the round-0 critical path.
            # The HAM clock-gate opens during the first real rounds; the
            # ACT-bound pipeline absorbs the short cold-PE ramp.
            warm = scp.tile([128, 512 * RW], F32, tag="score", name="warm")
            for i in range(3):
                nc.tensor.matmul(warm[:, :512], k_sb[0][:, 0:128],
                                 qh_sb[0][:, 0:512],
                                 start=(i == 0), stop=(i == 2))
            wp = ptp.tile([128, 512 * RW], F16, tag="p1", name="warmp")
            nc.scalar.activation(
                wp[:, :512], warm[:, :512],
                mybir.ActivationFunctionType.Exp, scale=ESC, bias=bias_t[:, :])

            ot_tiles = {}
            oseg_tiles = {}
            pend1, pend2 = [], []  # PV work lagged by 1 and 2 rounds

            def flush(items):
                for p1ref, i, u in items:
                    cid, kb = divmod(u, NKB)
                    s, c = divmod(cid, 4)
                    if kb == 0:
                        ot_tiles[cid] = otp.tile([128, 512], F32, tag="ot",
                                                 name=f"ot{cid}")
                    vsl = slice(kb * 128, (kb + 1) * 128)
                    psl = slice(i * 512, (i + 1) * 512)
                    nc.tensor.matmul(ot_tiles[cid][:, :], v_sb[s][:, vsl],
                                     p1ref[:, psl],
                                     start=(kb == 0), stop=(kb == NKB - 1))
                    if kb == NKB - 1:
                        if c == 0:
                            oseg_tiles[s] = osbp.tile(
                                [128, 4 * 512], F32, tag="oseg",
                                name=f"oseg{s}")
                        osl = slice(c * 512, (c + 1) * 512)
                        nc.vector.tensor_copy(oseg_tiles[s][:, osl],
                                              ot_tiles[cid][:, :])
                        if c == 3:
                            nc.sync.dma_start(
                                out_ap[:, s * 2048:(s + 1) * 2048],
                                oseg_tiles[s][:, :])

            for r in range((NUNIT + RW - 1) // RW):
                units = range(r * RW, min((r + 1) * RW, NUNIT))
                nu = len(units)
                score = scp.tile([128, 512 * RW], F32, tag="score",
                                 name=f"score{r}")
                for i, u in enumerate(units):
                    cid, kb = divmod(u, NKB)
                    s, c = divmod(cid, 4)
                    osl = slice(i * 512, (i + 1) * 512)
                    csl = slice(c * 512, (c + 1) * 512)
                    lhsT = k_sb[s][:, kb * 128:(kb + 1) * 128]
                    nc.tensor.matmul(score[:, osl], lhsT, qh_sb[s][:, csl],
                                     start=True, stop=True)
                nsl = slice(0, 512 * nu)
                p1 = ptp.tile([128, 512 * RW], F16, tag="p1", name=f"p1_{r}")
                nc.scalar.activation(
                    p1[:, nsl], score[:, nsl],
                    mybir.ActivationFunctionType.Exp, scale=ESC,
                    bias=bias_t[:, :])
                if r < 2:
                    # startup filler: PV work arrives only after the lag-2
                    # scores->exp pipeline; keep the PE from a long idle
                    # (HAM) with dummies aimed at an OT-pool slot.
                    fill = otp.tile([128, 512], F32, tag="ot", name=f"fill{r}")
                    for z in range(3):
                        nc.tensor.matmul(fill[:, :], k_sb[0][:, 0:128],
                                         qh_sb[0][:, 0:512],
                                         start=(z == 0), stop=(z == 2))
                flush(pend2)
                pend2 = pend1
                pend1 = [(p1, i, u) for i, u in enumerate(units)]
            flush(pend2)
            flush(pend1)

    nc.compile()
    return nc


def _prep_core(query, key, value, core):
    b, j = divmod(core, 4)
    segs = []
    for arr in (query, key, value):
        h0 = arr[b, :, j, :].reshape(4, SEG, D)
        h1 = arr[b, :, 4 + j, :].reshape(2, 4096, D)[:, 1::2, :]
        h2 = arr[b, 2::4, 8 + j, :][None]
        segs.append(np.concatenate([h0, h1, h2], axis=0))  # [7, 2048, 64]
    qs, ks, vs = segs
    # [64, NSEG*SEG] with col = s*SEG + p
    qt = (qs * QSC).transpose(2, 0, 1).reshape(D, NSEG * SEG)
    kt = (ks * QSC).transpose(2, 0, 1).reshape(D, NSEG * SEG)
    qh = qt.astype(np.float16)
    kh = kt.astype(np.float16)
    kl = (kt - kh).astype(np.float16)
    vv = vs * QSC  # [7, 2048, 64] f32, pre-scaled
    v1h = vv.astype(np.float16)
    v1l = (vv - v1h).astype(np.float16)
    # packed stationary blocks: [vh(64) | vl(ch 0-62) | ones] per k-block
    blk = np.empty((NSEG, SEG, 128), np.float16)
    blk[:, :, 0:64] = v1h
    blk[:, :, 64:127] = v1l[:, :, 0:63]
    blk[:, :, 127] = np.float16(256.0)
    vhl = blk.reshape(NSEG, NKB, 128, 128).transpose(2, 0, 1, 3).reshape(128, -1)
    return {
        "qhh": np.ascontiguousarray(np.concatenate([qh, qh], axis=0)),
        "khl": np.ascontiguousarray(np.concatenate([kh, kl], axis=0)),
        "vhl": np.ascontiguousarray(vhl),
    }


def _unshard(results, dtype):
    full = np.zeros((B, N, H, D), dtype)
    for core in range(8):
        b, j = divmod(core, 4)
        o = results[core]["out"].astype(np.float64)
        num = o[0:64].copy()
        num[0:63] += o[64:127]
        T = num / o[127:128]  # [64, 14336]
        h0 = T[:, :4 * SEG]
        full[b, :, j, :] = (h0 / (3.0 * h0.sum(1, keepdims=True))).T
        h1 = T[:, 4 * SEG:6 * SEG]
        h1 = h1 / (3.0 * h1.sum(1, keepdims=True))
        for g in range(2):
            full[b, g * 4096 + 1:(g + 1) * 4096:2, 4 + j, :] = \
                h1[:, g * SEG:(g + 1) * SEG].T
        h2 = T[:, 6 * SEG:]
        full[b, 2::4, 8 + j, :] = (h2 / (3.0 * h2.sum(1, keepdims=True))).T
    return full


def _ensure_axon_backend():
    """The bass PJRT path needs the axon/neuron jax backend. A harness may
    pin JAX_PLATFORMS=cpu for its reference; re-select axon if so."""
    import jax
    try:
        plat = jax.devices()[0].platform
    except Exception:
        plat = ""
    if plat not in ("axon", "neuron"):
        try:
            jax.config.update("jax_platforms", "axon,cpu")
            jax.devices()
        except Exception:
            pass


def kernel(query, key, value):
    _ensure_axon_backend()
    query = np.asarray(query, np.float32)
    key = np.asarray(key, np.float32)
    value = np.asarray(value, np.float32)
    assert query.shape == (B, N, H, D)

    if "nc" not in _CACHE:
        _CACHE["nc"] = _build_nc()
    nc = _CACHE["nc"]

    in_maps = [_prep_core(query, key, value, c) for c in range(8)]
    res = run_bass_kernel_spmd(nc, in_maps, core_ids=list(range(8)))
    LAST_RESULT["exec_time_ns"] = res.exec_time_ns
    return _unshard(res.results, query.dtype)


# revision 10
# speedup vs baseline: 1.0021x; 1.0021x over previous
"""Dilated attention (LongNet-style) Trainium2 kernel — v4 (fused PV).

Problem: query/key/value (2, 8192, 12, 64) f32. Three dilation groups
(segment lengths 2048/4096/8192, dilation 1/2/4, head slices 0:4/4:8/8:12).
Each group's gather produces independent dense attention over 2048-position
dilated segments; outputs are normalized per (batch, head, channel) by the
sum over all segment positions, and divided by num_groups.

Sharding: 8 cores = 2 batches x 4 "head columns". Core c owns batch c//4 and
heads {j, 4+j, 8+j} where j = c%4 -- exactly 7 dense 2048x2048x64 attention
units per core (4 + 2 + 1 segments), perfectly balanced, with all segments of
any (batch, head) on one core so normalization needs no cross-core traffic.

Precision (validated by numpy simulation of the exact arithmetic): the
x / x.sum normalization amplifies V-path errors ~140x but score/P-path
errors only ~8-15x. So K keeps an fp16 hi/lo pair packed along the
contraction dim (khl rows 0-63 = kh, 64-127 = kl; qhh rows = qh duplicated,
so scores = (kh+kl)^T qh in ONE fp16 matmul), Q and P are single fp16, and
V keeps an fp16 hi/lo pair.

v4 trick: a matmul's cost is its N (moving columns), not M, so the PV pair
+ denominator fuse into ONE matmul by packing the stationary operand as
  lhsT = [vh(ch 0-63) | vl(ch 0-62) | ones] (128 x 128):
output rows 0-63 = p1@vh, rows 64-126 = p1@vl, row 127 = softmax denom.
The HOST adds the hi/lo halves in f64. Channel 63 loses its lo-correction
(+~3e-3 error, channel-diagonal). Per 128x512 unit the PE now does just
2 matmuls (scores + fused PV) = 1296ns/round of 3, making the single ACT
exp pass (1530ns/round) the bottleneck. Sim end-to-end: ~6.6e-3 (thr 2e-2).

Device kernel (same program on all 8 cores, different data):
  - inputs: qhh/khl [128, 14336] fp16 (as above), vhl [128, 14336] fp16
    (the packed 128x128 stationary blocks per (seg, k-block)).
  - per (chunk, k-block) unit (28 q-chunks of 512 x 16 k-blocks):
      S^T = khl_blk.T @ qhh              (PE, 1 MM, PSUM f32)
      p1 = fp16(exp(S^T*0.125/65536 + ln64))   (ACT, PSUM -> SBUF)
      O'[128, 512] += vhl_blk.T @ p1     (PE, 1 MM, f32 PSUM, accum over kb)
  - O' copied PSUM->SBUF (DVE) into a per-segment staging tile, DMA'd to
    DRAM out [128, 14336] f32 once per segment (4 chunks).
Host: num = O'[0:64] (+= O'[64:127] for ch<63), den = O'[127], T = num/den,
then the group normalization (sum over positions) and /3, scattered into
the (2, 8192, 12, 64) output. Positions not in a dilated group stay zero.
"""

import os
import sys

if "/opt/trn_rl_repo" not in sys.path:
    sys.path.insert(0, "/opt/trn_rl_repo")
if "jax" not in sys.modules:
    os.environ.setdefault("JAX_PLATFORMS", "axon")

import numpy as np

import concourse.bass as bass  # noqa: F401
import concourse.mybir as mybir
import concourse.tile as tile
from concourse import bacc
from concourse.bass_utils import run_bass_kernel_spmd

F32 = mybir.dt.float32
F16 = mybir.dt.float16

B, N, H, D = 2, 8192, 12, 64
NSEG = 7           # segments per core
SEG = 2048         # dilated segment length
NCHUNK = NSEG * 4  # 512-wide q chunks per core
NKB = 16           # 128-row k blocks per segment
NUNIT = NCHUNK * NKB
RW = 3             # k-blocks per exp round (3 PSUM banks per ACT span)
QSC = np.float32(256.0)               # fp16 pre-scale for Q/K/V splits
ESC = float(0.125 / (256.0 * 256.0))  # exp scale: 1/sqrt(64) + descale
import math
PBIAS = float(math.log(64.0))         # exp bias: P *= 64, into fp16-normal range

_CACHE = {}
LAST_RESULT = {}


def _build_nc():
    nc = bacc.Bacc("TRN2", target_bir_lowering=False, debug=False,
                   enable_asserts=False, num_devices=8)
    qhh = nc.dram_tensor("qhh", [128, NSEG * SEG], F16, kind="ExternalInput")
    khl = nc.dram_tensor("khl", [128, NSEG * SEG], F16, kind="ExternalInput")
    vhl = nc.dram_tensor("vhl", [128, NSEG * NKB * 128], F16,
                         kind="ExternalInput")
    out = nc.dram_tensor("out", [128, NCHUNK * 512], F32, kind="ExternalOutput")
    qhh_ap, khl_ap, vhl_ap, out_ap = qhh.ap(), khl.ap(), vhl.ap(), out.ap()

    with tile.TileContext(nc) as tc:
        with (
            tc.tile_pool(name="inp", bufs=1) as inp,
            tc.tile_pool(name="pt", bufs=4) as ptp,
            tc.tile_pool(name="osb", bufs=2) as osbp,
            tc.tile_pool(name="score", bufs=2, space="PSUM") as scp,
            tc.tile_pool(name="ot", bufs=2, space="PSUM") as otp,
        ):
            bias_t = inp.tile([128, 1], F32, tag="bias", name="bias_t")
            nc.vector.memset(bias_t[:, :], PBIAS)

            qh_sb, k_sb, v_sb = [], [], []
            for s in range(NSEG):
                qh_sb.append(inp.tile([128, SEG], F16, tag=f"qh{s}",
                                      name=f"qh{s}"))
                k_sb.append(inp.tile([128, SEG], F16, tag=f"k{s}",
                                     name=f"k{s}"))
                v_sb.append(inp.tile([128, NKB * 128], F16, tag=f"v{s}",
                                     name=f"v{s}"))

            def dma_parts(t, ap_, s, w, nparts):
                step = w // nparts
                for z in range(nparts):
                    lo = z * step
                    nc.sync.dma_start(t[:, lo:lo + step],
                                      ap_[:, s * w + lo:s * w + lo + step])

            # DMA plan: round 0 needs only qhh seg-0 cols 0-511 and khl
            # k-block 0 (~164KB); emission order = HW queue-head order, so
            # the first-needed fine slices go out first, then the rest of
            # seg 0 in k-block-sized pieces, then segs 1-6 in bulk (needed
            # ~30us+ in; the 8.5us/seg steady-state pace dwarfs their DMA).
            dma_parts(qh_sb[0], qhh_ap, 0, SEG, 4)       # part 0 = cols 0-511
            dma_parts(k_sb[0], khl_ap, 0, SEG, 8)        # part 0 = kb 0-1
            dma_parts(v_sb[0], vhl_ap, 0, NKB * 128, 4)
            for s in range(1, NSEG):
                dma_parts(qh_sb[s], qhh_ap, s, SEG, 1)
                dma_parts(k_sb[s], khl_ap, s, SEG, 1)
                dma_parts(v_sb[s], vhl_ap, s, NKB * 128, 1)

            # Warm-up: replay the round-0 scores matmul into a scratch PSUM
            # slot as soon as the seg-0 part-0 DMAs land. Depending on input
            # tiles (not DVE-memset scratch) lets the first matmul fire ~3us
            # earlier than behind the DVE engine preamble, and the dummy exp
            # pulls the ~2.7us ACT table load off the round-0 critical path.
            # The HAM clock-gate opens during the first real rounds; the
            # ACT-bound pipeline absorbs the short cold-PE ramp.
            warm = scp.tile([128, 512 * RW], F32, tag="score", name="warm")
            for i in range(3):
                nc.tensor.matmul(warm[:, :512], k_sb[0][:, 0:128],
                                 qh_sb[0][:, 0:512],
                                 start=(i == 0), stop=(i == 2))
            wp = ptp.tile([128, 512 * RW], F16, tag="p1", name="warmp")
            nc.scalar.activation(
                wp[:, :512], warm[:, :512],
                mybir.ActivationFunctionType.Exp, scale=ESC, bias=bias_t[:, :])

            ot_tiles = {}
            oseg_tiles = {}
            pend1, pend2 = [], []  # PV work lagged by 1 and 2 rounds

            def flush(items):
                for p1ref, i, u in items:
                    cid, kb = divmod(u, NKB)
                    s, c = divmod(cid, 4)
                    if kb == 0:
                        ot_tiles[cid] = otp.tile([128, 512], F32, tag="ot",
                                                 name=f"ot{cid}")
                    vsl = slice(kb * 128, (kb + 1) * 128)
                    psl = slice(i * 512, (i + 1) * 512)
                    nc.tensor.matmul(ot_tiles[cid][:, :], v_sb[s][:, vsl],
                                     p1ref[:, psl],
                                     start=(kb == 0), stop=(kb == NKB - 1))
                    if kb == NKB - 1:
                        if c == 0:
                            oseg_tiles[s] = osbp.tile(
                                [128, 4 * 512], F32, tag="oseg",
                                name=f"oseg{s}")
                        osl = slice(c * 512, (c + 1) * 512)
                        nc.vector.tensor_copy(oseg_tiles[s][:, osl],
                                              ot_tiles[cid][:, :])
                        if c == 3:
                            nc.sync.dma_start(
                                out_ap[:, s * 2048:(s + 1) * 2048],
                                oseg_tiles[s][:, :])

            for r in range((NUNIT + RW - 1) // RW):
                units = range(r * RW, min((r + 1) * RW, NUNIT))
                nu = len(units)
                score = scp.tile([128, 512 * RW], F32, tag="score",
                                 name=f"score{r}")
                for i, u in enumerate(units):
                    cid, kb = divmod(u, NKB)
                    s, c = divmod(cid, 4)
                    osl = slice(i * 512, (i + 1) * 512)
                    csl = slice(c * 512, (c + 1) * 512)
                    lhsT = k_sb[s][:, kb * 128:(kb + 1) * 128]
                    nc.tensor.matmul(score[:, osl], lhsT, qh_sb[s][:, csl],
                                     start=True, stop=True)
                nsl = slice(0, 512 * nu)
                p1 = ptp.tile([128, 512 * RW], F16, tag="p1", name=f"p1_{r}")
                nc.scalar.activation(
                    p1[:, nsl], score[:, nsl],
                    mybir.ActivationFunctionType.Exp, scale=ESC,
                    bias=bias_t[:, :])
                if r < 2:
                    # startup filler: PV work arrives only after the lag-2
                    # scores->exp pipeline; keep the PE from a long idle
                    # (HAM) with dummies aimed at an OT-pool slot.
                    fill = otp.tile([128, 512], F32, tag="ot", name=f"fill{r}")
                    for z in range(3):
                        nc.tensor.matmul(fill[:, :], k_sb[0][:, 0:128],
                                         qh_sb[0][:, 0:512],
                                         start=(z == 0), stop=(z == 2))
                flush(pend2)
                pend2 = pend1
                pend1 = [(p1, i, u) for i, u in enumerate(units)]
            flush(pend2)
            flush(pend1)

    nc.compile()
    return nc


def _prep_core(query, key, value, core):
    b, j = divmod(core, 4)
    segs = []
    for arr in (query, key, value):
        h0 = arr[b, :, j, :].reshape(4, SEG, D)
        h1 = arr[b, :, 4 + j, :].reshape(2, 4096, D)[:, 1::2, :]
        h2 = arr[b, 2::4, 8 + j, :][None]
        segs.append(np.concatenate([h0, h1, h2], axis=0))  # [7, 2048, 64]
    qs, ks, vs = segs
    # [64, NSEG*SEG] with col = s*SEG + p
    qt = (qs * QSC).transpose(2, 0, 1).reshape(D, NSEG * SEG)
    kt = (ks * QSC).transpose(2, 0, 1).reshape(D, NSEG * SEG)
    qh = qt.astype(np.float16)
    kh = kt.astype(np.float16)
    kl = (kt - kh).astype(np.float16)
    vv = vs * QSC  # [7, 2048, 64] f32, pre-scaled
    v1h = vv.astype(np.float16)
    v1l = (vv - v1h).astype(np.float16)
    # packed stationary blocks: [vh(64) | vl(ch 0-62) | ones] per k-block
    blk = np.empty((NSEG, SEG, 128), np.float16)
    blk[:, :, 0:64] = v1h
    blk[:, :, 64:127] = v1l[:, :, 0:63]
    blk[:, :, 127] = np.float16(256.0)
    vhl = blk.reshape(NSEG, NKB, 128, 128).transpose(2, 0, 1, 3).reshape(128, -1)
    return {
        "qhh": np.ascontiguousarray(np.concatenate([qh, qh], axis=0)),
        "khl": np.ascontiguousarray(np.concatenate([kh, kl], axis=0)),
        "vhl": np.ascontiguousarray(vhl),
    }


def _unshard(results, dtype):
    full = np.zeros((B, N, H, D), dtype)
    for core in range(8):
        b, j = divmod(core, 4)
        o = results[core]["out"].astype(np.float64)
        num = o[0:64].copy()
        num[0:63] += o[64:127]
        T = num / o[127:128]  # [64, 14336]
        h0 = T[:, :4 * SEG]
        full[b, :, j, :] = (h0 / (3.0 * h0.sum(1, keepdims=True))).T
        h1 = T[:, 4 * SEG:6 * SEG]
        h1 = h1 / (3.0 * h1.sum(1, keepdims=True))
        for g in range(2):
            full[b, g * 4096 + 1:(g + 1) * 4096:2, 4 + j, :] = \
                h1[:, g * SEG:(g + 1) * SEG].T
        h2 = T[:, 6 * SEG:]
        full[b, 2::4, 8 + j, :] = (h2 / (3.0 * h2.sum(1, keepdims=True))).T
    return full


def _ensure_axon_backend():
    """The bass PJRT path needs the axon/neuron jax backend. A harness may
    pin JAX_PLATFORMS=cpu for its reference; re-select axon if so."""
    import jax
    try:
        plat = jax.devices()[0].platform
    except Exception:
        plat = ""
    if plat not in ("axon", "neuron"):
        try:
            jax.config.update("jax_platforms", "axon,cpu")
            jax.devices()
        except Exception:
            pass


def kernel(query, key, value):
    _ensure_axon_backend()
    query = np.asarray(query, np.float32)
    key = np.asarray(key, np.float32)
    value = np.asarray(value, np.float32)
    assert query.shape == (B, N, H, D)

    if "nc" not in _CACHE:
        _CACHE["nc"] = _build_nc()
    nc = _CACHE["nc"]

    in_maps = [_prep_core(query, key, value, c) for c in range(8)]
    res = run_bass_kernel_spmd(nc, in_maps, core_ids=list(range(8)))
    LAST_RESULT["exec_time_ns"] = res.exec_time_ns
    return _unshard(res.results, query.dtype)


# revision 13
# speedup vs baseline: 1.0040x; 1.0019x over previous
"""Dilated attention (LongNet-style) Trainium2 kernel — v4 (fused PV).

Problem: query/key/value (2, 8192, 12, 64) f32. Three dilation groups
(segment lengths 2048/4096/8192, dilation 1/2/4, head slices 0:4/4:8/8:12).
Each group's gather produces independent dense attention over 2048-position
dilated segments; outputs are normalized per (batch, head, channel) by the
sum over all segment positions, and divided by num_groups.

Sharding: 8 cores = 2 batches x 4 "head columns". Core c owns batch c//4 and
heads {j, 4+j, 8+j} where j = c%4 -- exactly 7 dense 2048x2048x64 attention
units per core (4 + 2 + 1 segments), perfectly balanced, with all segments of
any (batch, head) on one core so normalization needs no cross-core traffic.

Precision (validated by numpy simulation of the exact arithmetic): the
x / x.sum normalization amplifies V-path errors ~140x but score/P-path
errors only ~8-15x. So K keeps an fp16 hi/lo pair packed along the
contraction dim (khl rows 0-63 = kh, 64-127 = kl; qhh rows = qh duplicated,
so scores = (kh+kl)^T qh in ONE fp16 matmul), Q and P are single fp16, and
V keeps an fp16 hi/lo pair.

v4 trick: a matmul's cost is its N (moving columns), not M, so the PV pair
+ denominator fuse into ONE matmul by packing the stationary operand as
  lhsT = [vh(ch 0-63) | vl(ch 0-62) | ones] (128 x 128):
output rows 0-63 = p1@vh, rows 64-126 = p1@vl, row 127 = softmax denom.
The HOST adds the hi/lo halves in f64. Channel 63 loses its lo-correction
(+~3e-3 error, channel-diagonal). Per 128x512 unit the PE now does just
2 matmuls (scores + fused PV) = 1296ns/round of 3, making the single ACT
exp pass (1530ns/round) the bottleneck. Sim end-to-end: ~6.6e-3 (thr 2e-2).

Device kernel (same program on all 8 cores, different data):
  - inputs: qhh/khl [128, 14336] fp16 (as above), vhl [128, 14336] fp16
    (the packed 128x128 stationary blocks per (seg, k-block)).
  - per (chunk, k-block) unit (28 q-chunks of 512 x 16 k-blocks):
      S^T = khl_blk.T @ qhh              (PE, 1 MM, PSUM f32)
      p1 = fp16(exp(S^T*0.125/65536 + ln64))   (ACT, PSUM -> SBUF)
      O'[128, 512] += vhl_blk.T @ p1     (PE, 1 MM, f32 PSUM, accum over kb)
  - O' copied PSUM->SBUF (DVE) into a per-segment staging tile, DMA'd to
    DRAM out [128, 14336] f32 once per segment (4 chunks).
Host: num = O'[0:64] (+= O'[64:127] for ch<63), den = O'[127], T = num/den,
then the group normalization (sum over positions) and /3, scattered into
the (2, 8192, 12, 64) output. Positions not in a dilated group stay zero.
"""

import os
import sys

if "/opt/trn_rl_repo" not in sys.path:
    sys.path.insert(0, "/opt/trn_rl_repo")
if "jax" not in sys.modules:
    os.environ.setdefault("JAX_PLATFORMS", "axon")

import numpy as np

import concourse.bass as bass  # noqa: F401
import concourse.mybir as mybir
import concourse.tile as tile
from concourse import bacc
from concourse.bass_utils import run_bass_kernel_spmd

F32 = mybir.dt.float32
F16 = mybir.dt.float16

B, N, H, D = 2, 8192, 12, 64
NSEG = 7           # segments per core
SEG = 2048         # dilated segment length
NCHUNK = NSEG * 4  # 512-wide q chunks per core
NKB = 16           # 128-row k blocks per segment
NUNIT = NCHUNK * NKB
RW = 3             # k-blocks per exp round (3 PSUM banks per ACT span)
QSC = np.float32(256.0)               # fp16 pre-scale for Q/K/V splits
ESC = float(0.125 / (256.0 * 256.0))  # exp scale: 1/sqrt(64) + descale
import math
PBIAS = float(math.log(64.0))         # exp bias: P *= 64, into fp16-normal range

_CACHE = {}
LAST_RESULT = {}


def _build_nc():
    nc = bacc.Bacc("TRN2", target_bir_lowering=False, debug=False,
                   enable_asserts=False, num_devices=8)
    qhh = nc.dram_tensor("qhh", [128, NSEG * SEG], F16, kind="ExternalInput")
    khl = nc.dram_tensor("khl", [128, NSEG * SEG], F16, kind="ExternalInput")
    vhl = nc.dram_tensor("vhl", [128, NSEG * NKB * 128], F16,
                         kind="ExternalInput")
    out = nc.dram_tensor("out", [128, NCHUNK * 512], F32, kind="ExternalOutput")
    qhh_ap, khl_ap, vhl_ap, out_ap = qhh.ap(), khl.ap(), vhl.ap(), out.ap()

    with tile.TileContext(nc) as tc:
        with (
            tc.tile_pool(name="inp", bufs=1) as inp,
            tc.tile_pool(name="pt", bufs=4) as ptp,
            tc.tile_pool(name="osb", bufs=2) as osbp,
            tc.tile_pool(name="score", bufs=2, space="PSUM") as scp,
            tc.tile_pool(name="ot", bufs=2, space="PSUM") as otp,
        ):
            bias_t = inp.tile([128, 1], F32, tag="bias", name="bias_t")
            nc.vector.memset(bias_t[:, :], PBIAS)

            qh_sb, k_sb, v_sb = [], [], []
            for s in range(NSEG):
                qh_sb.append(inp.tile([128, SEG], F16, tag=f"qh{s}",
                                      name=f"qh{s}"))
                k_sb.append(inp.tile([128, SEG], F16, tag=f"k{s}",
                                     name=f"k{s}"))
                v_sb.append(inp.tile([128, NKB * 128], F16, tag=f"v{s}",
                                     name=f"v{s}"))

            # DMA plan. Constraints measured from traces: each dma_start
            # occupies the sync engine ~610ns (serial issue), the engine is
            # only free at ~7.2us (engine preamble), and a single HW queue
            # moves ~24-35GB/s. So the slices round 0 strictly needs (qhh
            # seg-0 cols 0-511, khl k-blocks 0-2) are issued first as small
            # pieces across parallel queues, then the rest ordered by first
            # use. Bulk segs 1-6 are needed ~30us+ in. (ACT is also a HWDGE
            # engine and free earlier, but ACT-issued DMAs crash this
            # runtime path.)
            def slices(eng, t, ap_, s, w, cuts):
                for lo, hi in cuts:
                    eng.dma_start(t[:, lo:hi], ap_[:, s * w + lo:s * w + hi])

            slices(nc.sync, qh_sb[0], qhh_ap, 0, SEG,
                   [(0, 256), (256, 512)])
            slices(nc.sync, k_sb[0], khl_ap, 0, SEG,
                   [(0, 128), (128, 384)])
            slices(nc.sync, k_sb[0], khl_ap, 0, SEG,
                   [(384, 1024), (1024, 2048)])
            slices(nc.sync, qh_sb[0], qhh_ap, 0, SEG, [(512, 2048)])
            slices(nc.sync, v_sb[0], vhl_ap, 0, NKB * 128,
                   [(0, 1024), (1024, 2048)])
            for s in range(1, NSEG):
                slices(nc.sync, qh_sb[s], qhh_ap, s, SEG, [(0, SEG)])
                slices(nc.sync, k_sb[s], khl_ap, s, SEG, [(0, SEG)])
                slices(nc.sync, v_sb[s], vhl_ap, s, NKB * 128,
                       [(0, NKB * 128)])

            # Warm-up: replay the round-0 scores matmul into a scratch PSUM
            # slot as soon as the seg-0 part-0 DMAs land. Depending on input
            # tiles (not DVE-memset scratch) lets the first matmul fire ~3us
            # earlier than behind the DVE engine preamble, and the dummy exp
            # pulls the ~2.7us ACT table load off the round-0 critical path.
            # The HAM clock-gate opens during the first real rounds; the
            # ACT-bound pipeline absorbs the short cold-PE ramp.
            warm = scp.tile([128, 512 * RW], F32, tag="score", name="warm")
            for i in range(3):
                nc.tensor.matmul(warm[:, :512], k_sb[0][:, 0:128],
                                 qh_sb[0][:, 0:512],
                                 start=(i == 0), stop=(i == 2))
            wp = ptp.tile([128, 512 * RW], F16, tag="p1", name="warmp")
            nc.scalar.activation(
                wp[:, :512], warm[:, :512],
                mybir.ActivationFunctionType.Exp, scale=ESC, bias=bias_t[:, :])

            ot_tiles = {}
            oseg_tiles = {}
            pend1, pend2 = [], []  # PV work lagged by 1 and 2 rounds

            def flush(items):
                for p1ref, i, u in items:
                    cid, kb = divmod(u, NKB)
                    s, c = divmod(cid, 4)
                    if kb == 0:
                        ot_tiles[cid] = otp.tile([128, 512], F32, tag="ot",
                                                 name=f"ot{cid}")
                    vsl = slice(kb * 128, (kb + 1) * 128)
                    psl = slice(i * 512, (i + 1) * 512)
                    nc.tensor.matmul(ot_tiles[cid][:, :], v_sb[s][:, vsl],
                                     p1ref[:, psl],
                                     start=(kb == 0), stop=(kb == NKB - 1))
                    if kb == NKB - 1:
                        if c == 0:
                            oseg_tiles[s] = osbp.tile(
                                [128, 4 * 512], F32, tag="oseg",
                                name=f"oseg{s}")
                        osl = slice(c * 512, (c + 1) * 512)
                        nc.vector.tensor_copy(oseg_tiles[s][:, osl],
                                              ot_tiles[cid][:, :])
                        if c == 3:
                            nc.sync.dma_start(
                                out_ap[:, s * 2048:(s + 1) * 2048],
                                oseg_tiles[s][:, :])

            for r in range((NUNIT + RW - 1) // RW):
                units = range(r * RW, min((r + 1) * RW, NUNIT))
                nu = len(units)
                score = scp.tile([128, 512 * RW], F32, tag="score",
                                 name=f"score{r}")
                for i, u in enumerate(units):
                    cid, kb = divmod(u, NKB)
                    s, c = divmod(cid, 4)
                    osl = slice(i * 512, (i + 1) * 512)
                    csl = slice(c * 512, (c + 1) * 512)
                    lhsT = k_sb[s][:, kb * 128:(kb + 1) * 128]
                    nc.tensor.matmul(score[:, osl], lhsT, qh_sb[s][:, csl],
                                     start=True, stop=True)
                nsl = slice(0, 512 * nu)
                p1 = ptp.tile([128, 512 * RW], F16, tag="p1", name=f"p1_{r}")
                nc.scalar.activation(
                    p1[:, nsl], score[:, nsl],
                    mybir.ActivationFunctionType.Exp, scale=ESC,
                    bias=bias_t[:, :])
                if r < 2:
                    # startup filler: PV work arrives only after the lag-2
                    # scores->exp pipeline; keep the PE from a long idle
                    # (HAM) with dummies aimed at an OT-pool slot.
                    fill = otp.tile([128, 512], F32, tag="ot", name=f"fill{r}")
                    for z in range(3):
                        nc.tensor.matmul(fill[:, :], k_sb[0][:, 0:128],
                                         qh_sb[0][:, 0:512],
                                         start=(z == 0), stop=(z == 2))
                flush(pend2)
                pend2 = pend1
                pend1 = [(p1, i, u) for i, u in enumerate(units)]
            flush(pend2)
            flush(pend1)

    nc.compile()
    return nc


def _prep_core(query, key, value, core):
    b, j = divmod(core, 4)
    segs = []
    for arr in (query, key, value):
        h0 = arr[b, :, j, :].reshape(4, SEG, D)
        h1 = arr[b, :, 4 + j, :].reshape(2, 4096, D)[:, 1::2, :]
        h2 = arr[b, 2::4, 8 + j, :][None]
        segs.append(np.concatenate([h0, h1, h2], axis=0))  # [7, 2048, 64]
    qs, ks, vs = segs
    # [64, NSEG*SEG] with col = s*SEG + p
    qt = (qs * QSC).transpose(2, 0, 1).reshape(D, NSEG * SEG)
    kt = (ks * QSC).transpose(2, 0, 1).reshape(D, NSEG * SEG)
    qh = qt.astype(np.float16)
    kh = kt.astype(np.float16)
    kl = (kt - kh).astype(np.float16)
    vv = vs * QSC  # [7, 2048, 64] f32, pre-scaled
    v1h = vv.astype(np.float16)
    v1l = (vv - v1h).astype(np.float16)
    # packed stationary blocks: [vh(64) | vl(ch 0-62) | ones] per k-block
    blk = np.empty((NSEG, SEG, 128), np.float16)
    blk[:, :, 0:64] = v1h
    blk[:, :, 64:127] = v1l[:, :, 0:63]
    blk[:, :, 127] = np.float16(256.0)
    vhl = blk.reshape(NSEG, NKB, 128, 128).transpose(2, 0, 1, 3).reshape(128, -1)
    return {
        "qhh": np.ascontiguousarray(np.concatenate([qh, qh], axis=0)),
        "khl": np.ascontiguousarray(np.concatenate([kh, kl], axis=0)),
        "vhl": np.ascontiguousarray(vhl),
    }


def _unshard(results, dtype):
    full = np.zeros((B, N, H, D), dtype)
    for core in range(8):
        b, j = divmod(core, 4)
        o = results[core]["out"].astype(np.float64)
        num = o[0:64].copy()
        num[0:63] += o[64:127]
        T = num / o[127:128]  # [64, 14336]
        h0 = T[:, :4 * SEG]
        full[b, :, j, :] = (h0 / (3.0 * h0.sum(1, keepdims=True))).T
        h1 = T[:, 4 * SEG:6 * SEG]
        h1 = h1 / (3.0 * h1.sum(1, keepdims=True))
        for g in range(2):
            full[b, g * 4096 + 1:(g + 1) * 4096:2, 4 + j, :] = \
                h1[:, g * SEG:(g + 1) * SEG].T
        h2 = T[:, 6 * SEG:]
        full[b, 2::4, 8 + j, :] = (h2 / (3.0 * h2.sum(1, keepdims=True))).T
    return full


def _ensure_axon_backend():
    """The bass PJRT path needs the axon/neuron jax backend. A harness may
    pin JAX_PLATFORMS=cpu for its reference; re-select axon if so."""
    import jax
    try:
        plat = jax.devices()[0].platform
    except Exception:
        plat = ""
    if plat not in ("axon", "neuron"):
        try:
            jax.config.update("jax_platforms", "axon,cpu")
            jax.devices()
        except Exception:
            pass


def kernel(query, key, value):
    _ensure_axon_backend()
    query = np.asarray(query, np.float32)
    key = np.asarray(key, np.float32)
    value = np.asarray(value, np.float32)
    assert query.shape == (B, N, H, D)

    if "nc" not in _CACHE:
        _CACHE["nc"] = _build_nc()
    nc = _CACHE["nc"]

    in_maps = [_prep_core(query, key, value, c) for c in range(8)]
    res = run_bass_kernel_spmd(nc, in_maps, core_ids=list(range(8)))
    LAST_RESULT["exec_time_ns"] = res.exec_time_ns
    return _unshard(res.results, query.dtype)


# revision 15
# speedup vs baseline: 1.0066x; 1.0026x over previous
"""Dilated attention (LongNet-style) Trainium2 kernel — v4 (fused PV).

Problem: query/key/value (2, 8192, 12, 64) f32. Three dilation groups
(segment lengths 2048/4096/8192, dilation 1/2/4, head slices 0:4/4:8/8:12).
Each group's gather produces independent dense attention over 2048-position
dilated segments; outputs are normalized per (batch, head, channel) by the
sum over all segment positions, and divided by num_groups.

Sharding: 8 cores = 2 batches x 4 "head columns". Core c owns batch c//4 and
heads {j, 4+j, 8+j} where j = c%4 -- exactly 7 dense 2048x2048x64 attention
units per core (4 + 2 + 1 segments), perfectly balanced, with all segments of
any (batch, head) on one core so normalization needs no cross-core traffic.

Precision (validated by numpy simulation of the exact arithmetic): the
x / x.sum normalization amplifies V-path errors ~140x but score/P-path
errors only ~8-15x. So K keeps an fp16 hi/lo pair packed along the
contraction dim (khl rows 0-63 = kh, 64-127 = kl; qhh rows = qh duplicated,
so scores = (kh+kl)^T qh in ONE fp16 matmul), Q and P are single fp16, and
V keeps an fp16 hi/lo pair.

v4 trick: a matmul's cost is its N (moving columns), not M, so the PV pair
+ denominator fuse into ONE matmul by packing the stationary operand as
  lhsT = [vh(ch 0-63) | vl(ch 0-62) | ones] (128 x 128):
output rows 0-63 = p1@vh, rows 64-126 = p1@vl, row 127 = softmax denom.
The HOST adds the hi/lo halves in f64. Channel 63 loses its lo-correction
(+~3e-3 error, channel-diagonal). Per 128x512 unit the PE now does just
2 matmuls (scores + fused PV) = 1296ns/round of 3, making the single ACT
exp pass (1530ns/round) the bottleneck. Sim end-to-end: ~6.6e-3 (thr 2e-2).

Device kernel (same program on all 8 cores, different data):
  - inputs: qhh/khl [128, 14336] fp16 (as above), vhl [128, 14336] fp16
    (the packed 128x128 stationary blocks per (seg, k-block)).
  - per (chunk, k-block) unit (28 q-chunks of 512 x 16 k-blocks):
      S^T = khl_blk.T @ qhh              (PE, 1 MM, PSUM f32)
      p1 = fp16(exp(S^T*0.125/65536 + ln64))   (ACT, PSUM -> SBUF)
      O'[128, 512] += vhl_blk.T @ p1     (PE, 1 MM, f32 PSUM, accum over kb)
  - O' copied PSUM->SBUF (DVE) into a per-segment staging tile, DMA'd to
    DRAM out [128, 14336] f32 once per segment (4 chunks).
Host: num = O'[0:64] (+= O'[64:127] for ch<63), den = O'[127], T = num/den,
then the group normalization (sum over positions) and /3, scattered into
the (2, 8192, 12, 64) output. Positions not in a dilated group stay zero.
"""

import os
import sys

if "/opt/trn_rl_repo" not in sys.path:
    sys.path.insert(0, "/opt/trn_rl_repo")
if "jax" not in sys.modules:
    os.environ.setdefault("JAX_PLATFORMS", "axon")

import numpy as np

import concourse.bass as bass  # noqa: F401
import concourse.mybir as mybir
import concourse.tile as tile
from concourse import bacc
from concourse.bass_utils import run_bass_kernel_spmd

F32 = mybir.dt.float32
F16 = mybir.dt.float16

B, N, H, D = 2, 8192, 12, 64
NSEG = 7           # segments per core
SEG = 2048         # dilated segment length
NCHUNK = NSEG * 4  # 512-wide q chunks per core
NKB = 16           # 128-row k blocks per segment
NUNIT = NCHUNK * NKB
RW = 3             # k-blocks per exp round (3 PSUM banks per ACT span)
QSC = np.float32(256.0)               # fp16 pre-scale for Q/K/V splits
ESC = float(0.125 / (256.0 * 256.0))  # exp scale: 1/sqrt(64) + descale
import math
PBIAS = float(math.log(64.0))         # exp bias: P *= 64, into fp16-normal range

_CACHE = {}
LAST_RESULT = {}


def _build_nc():
    nc = bacc.Bacc("TRN2", target_bir_lowering=False, debug=False,
                   enable_asserts=False, num_devices=8)
    qhh = nc.dram_tensor("qhh", [128, NSEG * SEG], F16, kind="ExternalInput")
    khl = nc.dram_tensor("khl", [128, NSEG * SEG], F16, kind="ExternalInput")
    vhl = nc.dram_tensor("vhl", [128, NSEG * NKB * 128], F16,
                         kind="ExternalInput")
    out = nc.dram_tensor("out", [128, NCHUNK * 512], F32, kind="ExternalOutput")
    qhh_ap, khl_ap, vhl_ap, out_ap = qhh.ap(), khl.ap(), vhl.ap(), out.ap()

    with tile.TileContext(nc) as tc:
        with (
            tc.tile_pool(name="inp", bufs=1) as inp,
            tc.tile_pool(name="pt", bufs=4) as ptp,
            tc.tile_pool(name="osb", bufs=2) as osbp,
            tc.tile_pool(name="score", bufs=2, space="PSUM") as scp,
            tc.tile_pool(name="ot", bufs=2, space="PSUM") as otp,
        ):
            bias_t = inp.tile([128, 1], F32, tag="bias", name="bias_t")
            nc.vector.memset(bias_t[:, :], PBIAS)

            qh_sb, k_sb, v_sb = [], [], []
            for s in range(NSEG):
                qh_sb.append(inp.tile([128, SEG], F16, tag=f"qh{s}",
                                      name=f"qh{s}"))
                k_sb.append(inp.tile([128, SEG], F16, tag=f"k{s}",
                                     name=f"k{s}"))
                v_sb.append(inp.tile([128, NKB * 128], F16, tag=f"v{s}",
                                     name=f"v{s}"))

            # DMA plan. Constraints measured from traces: each dma_start
            # occupies the sync engine ~610ns (serial issue), the engine is
            # only free at ~7.2us (engine preamble), and a single HW queue
            # moves ~24-35GB/s. So the slices round 0 strictly needs (qhh
            # seg-0 cols 0-511, khl k-blocks 0-2) are issued first as small
            # pieces across parallel queues, then the rest ordered by first
            # use. Bulk segs 1-6 are needed ~30us+ in. (ACT is also a HWDGE
            # engine and free earlier, but ACT-issued DMAs crash this
            # runtime path.)
            def slices(eng, t, ap_, s, w, cuts):
                for lo, hi in cuts:
                    eng.dma_start(t[:, lo:hi], ap_[:, s * w + lo:s * w + hi])

            slices(nc.sync, qh_sb[0], qhh_ap, 0, SEG,
                   [(0, 256), (256, 512)])
            slices(nc.sync, k_sb[0], khl_ap, 0, SEG,
                   [(0, 128), (128, 384)])
            slices(nc.sync, k_sb[0], khl_ap, 0, SEG,
                   [(384, 1024), (1024, 2048)])
            slices(nc.sync, qh_sb[0], qhh_ap, 0, SEG, [(512, 2048)])
            slices(nc.sync, v_sb[0], vhl_ap, 0, NKB * 128,
                   [(0, 1024), (1024, 2048)])
            for s in range(1, NSEG):
                slices(nc.sync, qh_sb[s], qhh_ap, s, SEG, [(0, SEG)])
                slices(nc.sync, k_sb[s], khl_ap, s, SEG, [(0, SEG)])
                slices(nc.sync, v_sb[s], vhl_ap, s, NKB * 128,
                       [(0, NKB * 128)])

            # Warm-up: a few dummy matmuls on DVE-memset scratch (ready at
            # ~7.8us, before the first input DMA lands) keep the PE busy and
            # complete a PSUM slice fast so the dummy exp — and with it the
            # ~2.7us ACT table load — runs off the round-0 critical path.
            # The HAM clock-gate opens during the first real rounds; the
            # ACT-bound pipeline absorbs the short cold-PE ramp.
            wsrc = inp.tile([128, 128], F16, tag="wsrc", name="wsrc")
            wjunk = inp.tile([128, 512], F16, tag="wjunk", name="wjunk")
            nc.vector.memset(wsrc[:, :], 0.01)
            nc.vector.memset(wjunk[:, :], 0.01)
            warm = scp.tile([128, 512 * RW], F32, tag="score", name="warm")
            for i in range(3):
                nc.tensor.matmul(warm[:, :512], wsrc[:, :], wjunk[:, :],
                                 start=(i == 0), stop=(i == 2))
            wp = ptp.tile([128, 512 * RW], F16, tag="p1", name="warmp")
            nc.scalar.activation(
                wp[:, :512], warm[:, :512],
                mybir.ActivationFunctionType.Exp, scale=ESC, bias=bias_t[:, :])

            ot_tiles = {}
            oseg_tiles = {}
            pend1, pend2 = [], []  # PV work lagged by 1 and 2 rounds

            def flush(items):
                for p1ref, i, u in items:
                    cid, kb = divmod(u, NKB)
                    s, c = divmod(cid, 4)
                    if kb == 0:
                        ot_tiles[cid] = otp.tile([128, 512], F32, tag="ot",
                                                 name=f"ot{cid}")
                    vsl = slice(kb * 128, (kb + 1) * 128)
                    psl = slice(i * 512, (i + 1) * 512)
                    nc.tensor.matmul(ot_tiles[cid][:, :], v_sb[s][:, vsl],
                                     p1ref[:, psl],
                                     start=(kb == 0), stop=(kb == NKB - 1))
                    if kb == NKB - 1:
                        if c == 0:
                            oseg_tiles[s] = osbp.tile(
                                [128, 4 * 512], F32, tag="oseg",
                                name=f"oseg{s}")
                        osl = slice(c * 512, (c + 1) * 512)
                        nc.vector.tensor_copy(oseg_tiles[s][:, osl],
                                              ot_tiles[cid][:, :])
                        if c == 3:
                            nc.sync.dma_start(
                                out_ap[:, s * 2048:(s + 1) * 2048],
                                oseg_tiles[s][:, :])

            for r in range((NUNIT + RW - 1) // RW):
                units = range(r * RW, min((r + 1) * RW, NUNIT))
                nu = len(units)
                score = scp.tile([128, 512 * RW], F32, tag="score",
                                 name=f"score{r}")
                for i, u in enumerate(units):
                    cid, kb = divmod(u, NKB)
                    s, c = divmod(cid, 4)
                    osl = slice(i * 512, (i + 1) * 512)
                    csl = slice(c * 512, (c + 1) * 512)
                    lhsT = k_sb[s][:, kb * 128:(kb + 1) * 128]
                    nc.tensor.matmul(score[:, osl], lhsT, qh_sb[s][:, csl],
                                     start=True, stop=True)
                nsl = slice(0, 512 * nu)
                p1 = ptp.tile([128, 512 * RW], F16, tag="p1", name=f"p1_{r}")
                nc.scalar.activation(
                    p1[:, nsl], score[:, nsl],
                    mybir.ActivationFunctionType.Exp, scale=ESC,
                    bias=bias_t[:, :])
                if r < 2:
                    # startup filler: PV work arrives only after the lag-2
                    # scores->exp pipeline; keep the PE from a long idle
                    # (HAM) with dummies aimed at an OT-pool slot.
                    fill = otp.tile([128, 512], F32, tag="ot", name=f"fill{r}")
                    for z in range(3):
                        nc.tensor.matmul(fill[:, :], wsrc[:, :], wjunk[:, :],
                                         start=(z == 0), stop=(z == 2))
                flush(pend2)
                pend2 = pend1
                pend1 = [(p1, i, u) for i, u in enumerate(units)]
            flush(pend2)
            flush(pend1)

    nc.compile()
    return nc


def _prep_core(query, key, value, core):
    b, j = divmod(core, 4)
    segs = []
    for arr in (query, key, value):
        h0 = arr[b, :, j, :].reshape(4, SEG, D)
        h1 = arr[b, :, 4 + j, :].reshape(2, 4096, D)[:, 1::2, :]
        h2 = arr[b, 2::4, 8 + j, :][None]
        segs.append(np.concatenate([h0, h1, h2], axis=0))  # [7, 2048, 64]
    qs, ks, vs = segs
    # [64, NSEG*SEG] with col = s*SEG + p
    qt = (qs * QSC).transpose(2, 0, 1).reshape(D, NSEG * SEG)
    kt = (ks * QSC).transpose(2, 0, 1).reshape(D, NSEG * SEG)
    qh = qt.astype(np.float16)
    kh = kt.astype(np.float16)
    kl = (kt - kh).astype(np.float16)
    vv = vs * QSC  # [7, 2048, 64] f32, pre-scaled
    v1h = vv.astype(np.float16)
    v1l = (vv - v1h).astype(np.float16)
    # packed stationary blocks: [vh(64) | vl(ch 0-62) | ones] per k-block
    blk = np.empty((NSEG, SEG, 128), np.float16)
    blk[:, :, 0:64] = v1h
    blk[:, :, 64:127] = v1l[:, :, 0:63]
    blk[:, :, 127] = np.float16(256.0)
    vhl = blk.reshape(NSEG, NKB, 128, 128).transpose(2, 0, 1, 3).reshape(128, -1)
    return {
        "qhh": np.ascontiguousarray(np.concatenate([qh, qh], axis=0)),
        "khl": np.ascontiguousarray(np.concatenate([kh, kl], axis=0)),
        "vhl": np.ascontiguousarray(vhl),
    }


def _unshard(results, dtype):
    full = np.zeros((B, N, H, D), dtype)
    for core in range(8):
        b, j = divmod(core, 4)
        o = results[core]["out"].astype(np.float64)
        num = o[0:64].copy()
        num[0:63] += o[64:127]
        T = num / o[127:128]  # [64, 14336]
        h0 = T[:, :4 * SEG]
        full[b, :, j, :] = (h0 / (3.0 * h0.sum(1, keepdims=True))).T
        h1 = T[:, 4 * SEG:6 * SEG]
        h1 = h1 / (3.0 * h1.sum(1, keepdims=True))
        for g in range(2):
            full[b, g * 4096 + 1:(g + 1) * 4096:2, 4 + j, :] = \
                h1[:, g * SEG:(g + 1) * SEG].T
        h2 = T[:, 6 * SEG:]
        full[b, 2::4, 8 + j, :] = (h2 / (3.0 * h2.sum(1, keepdims=True))).T
    return full


def _ensure_axon_backend():
    """The bass PJRT path needs the axon/neuron jax backend. A harness may
    pin JAX_PLATFORMS=cpu for its reference; re-select axon if so."""
    import jax
    try:
        plat = jax.devices()[0].platform
    except Exception:
        plat = ""
    if plat not in ("axon", "neuron"):
        try:
            jax.config.update("jax_platforms", "axon,cpu")
            jax.devices()
        except Exception:
            pass


def kernel(query, key, value):
    _ensure_axon_backend()
    query = np.asarray(query, np.float32)
    key = np.asarray(key, np.float32)
    value = np.asarray(value, np.float32)
    assert query.shape == (B, N, H, D)

    if "nc" not in _CACHE:
        _CACHE["nc"] = _build_nc()
    nc = _CACHE["nc"]

    in_maps = [_prep_core(query, key, value, c) for c in range(8)]
    res = run_bass_kernel_spmd(nc, in_maps, core_ids=list(range(8)))
    LAST_RESULT["exec_time_ns"] = res.exec_time_ns
    return _unshard(res.results, query.dtype)


# revision 16
# speedup vs baseline: 1.0115x; 1.0049x over previous
"""Dilated attention (LongNet-style) Trainium2 kernel — v4 (fused PV).

Problem: query/key/value (2, 8192, 12, 64) f32. Three dilation groups
(segment lengths 2048/4096/8192, dilation 1/2/4, head slices 0:4/4:8/8:12).
Each group's gather produces independent dense attention over 2048-position
dilated segments; outputs are normalized per (batch, head, channel) by the
sum over all segment positions, and divided by num_groups.

Sharding: 8 cores = 2 batches x 4 "head columns". Core c owns batch c//4 and
heads {j, 4+j, 8+j} where j = c%4 -- exactly 7 dense 2048x2048x64 attention
units per core (4 + 2 + 1 segments), perfectly balanced, with all segments of
any (batch, head) on one core so normalization needs no cross-core traffic.

Precision (validated by numpy simulation of the exact arithmetic): the
x / x.sum normalization amplifies V-path errors ~140x but score/P-path
errors only ~8-15x. So K keeps an fp16 hi/lo pair packed along the
contraction dim (khl rows 0-63 = kh, 64-127 = kl; qhh rows = qh duplicated,
so scores = (kh+kl)^T qh in ONE fp16 matmul), Q and P are single fp16, and
V keeps an fp16 hi/lo pair.

v4 trick: a matmul's cost is its N (moving columns), not M, so the PV pair
+ denominator fuse into ONE matmul by packing the stationary operand as
  lhsT = [vh(ch 0-63) | vl(ch 0-62) | ones] (128 x 128):
output rows 0-63 = p1@vh, rows 64-126 = p1@vl, row 127 = softmax denom.
The HOST adds the hi/lo halves in f64. Channel 63 loses its lo-correction
(+~3e-3 error, channel-diagonal). Per 128x512 unit the PE now does just
2 matmuls (scores + fused PV) = 1296ns/round of 3, making the single ACT
exp pass (1530ns/round) the bottleneck. Sim end-to-end: ~6.6e-3 (thr 2e-2).

Device kernel (same program on all 8 cores, different data):
  - inputs: qhh/khl [128, 14336] fp16 (as above), vhl [128, 14336] fp16
    (the packed 128x128 stationary blocks per (seg, k-block)).
  - per (chunk, k-block) unit (28 q-chunks of 512 x 16 k-blocks):
      S^T = khl_blk.T @ qhh              (PE, 1 MM, PSUM f32)
      p1 = fp16(exp(S^T*0.125/65536 + ln64))   (ACT, PSUM -> SBUF)
      O'[128, 512] += vhl_blk.T @ p1     (PE, 1 MM, f32 PSUM, accum over kb)
  - O' copied PSUM->SBUF (DVE) into a per-segment staging tile, DMA'd to
    DRAM out [128, 14336] f32 once per segment (4 chunks).
Host: num = O'[0:64] (+= O'[64:127] for ch<63), den = O'[127], T = num/den,
then the group normalization (sum over positions) and /3, scattered into
the (2, 8192, 12, 64) output. Positions not in a dilated group stay zero.
"""

import os
import sys

if "/opt/trn_rl_repo" not in sys.path:
    sys.path.insert(0, "/opt/trn_rl_repo")
if "jax" not in sys.modules:
    os.environ.setdefault("JAX_PLATFORMS", "axon")

import numpy as np

import concourse.bass as bass  # noqa: F401
import concourse.mybir as mybir
import concourse.tile as tile
from concourse import bacc
from concourse.bass_utils import run_bass_kernel_spmd

F32 = mybir.dt.float32
F16 = mybir.dt.float16

B, N, H, D = 2, 8192, 12, 64
NSEG = 7           # segments per core
SEG = 2048         # dilated segment length
NCHUNK = NSEG * 4  # 512-wide q chunks per core
NKB = 16           # 128-row k blocks per segment
NUNIT = NCHUNK * NKB
RW = 3             # k-blocks per exp round (3 PSUM banks per ACT span)
QSC = np.float32(256.0)               # fp16 pre-scale for Q/K/V splits
ESC = float(0.125 / (256.0 * 256.0))  # exp scale: 1/sqrt(64) + descale
import math
PBIAS = float(math.log(64.0))         # exp bias: P *= 64, into fp16-normal range

_CACHE = {}
LAST_RESULT = {}


def _build_nc():
    nc = bacc.Bacc("TRN2", target_bir_lowering=False, debug=False,
                   enable_asserts=False, num_devices=8)
    qhh = nc.dram_tensor("qhh", [128, NSEG * SEG], F16, kind="ExternalInput")
    khl = nc.dram_tensor("khl", [128, NSEG * SEG], F16, kind="ExternalInput")
    vhl = nc.dram_tensor("vhl", [128, NSEG * NKB * 128], F16,
                         kind="ExternalInput")
    out = nc.dram_tensor("out", [128, NCHUNK * 512], F32, kind="ExternalOutput")
    qhh_ap, khl_ap, vhl_ap, out_ap = qhh.ap(), khl.ap(), vhl.ap(), out.ap()

    with tile.TileContext(nc) as tc:
        with (
            tc.tile_pool(name="inp", bufs=1) as inp,
            tc.tile_pool(name="pt", bufs=4) as ptp,
            tc.tile_pool(name="osb", bufs=2) as osbp,
            tc.tile_pool(name="score", bufs=2, space="PSUM") as scp,
            tc.tile_pool(name="ot", bufs=2, space="PSUM") as otp,
        ):
            bias_t = inp.tile([128, 1], F32, tag="bias", name="bias_t")
            nc.vector.memset(bias_t[:, :], PBIAS)

            # Minimal warm-up: 3 dummy matmuls complete a PSUM slice fast so
            # the dummy exp (and with it the ~2.7us ACT table load) fires
            # early, overlapping the first input DMAs. The HAM clock-gate
            # opens during the first real rounds (ACT-bound pipeline absorbs
            # the short cold-PE ramp).
            wsrc = inp.tile([128, 128], F16, tag="wsrc", name="wsrc")
            wjunk = inp.tile([128, 512], F16, tag="wjunk", name="wjunk")
            nc.vector.memset(wsrc[:, :], 0.01)
            nc.vector.memset(wjunk[:, :], 0.01)
            warm = scp.tile([128, 512 * RW], F32, tag="score", name="warm")
            for i in range(3):
                nc.tensor.matmul(warm[:, :512], wsrc[:, :], wjunk[:, :],
                                 start=(i == 0), stop=(i == 2))
            wp = ptp.tile([128, 512 * RW], F16, tag="p1", name="warmp")
            nc.scalar.activation(
                wp[:, :512], warm[:, :512],
                mybir.ActivationFunctionType.Exp, scale=ESC, bias=bias_t[:, :])

            qh_sb, k_sb, v_sb = [], [], []
            for s in range(NSEG):
                qh = inp.tile([128, SEG], F16, tag=f"qh{s}", name=f"qh{s}")
                kk = inp.tile([128, SEG], F16, tag=f"k{s}", name=f"k{s}")
                vv = inp.tile([128, NKB * 128], F16, tag=f"v{s}", name=f"v{s}")
                # split the first segment's transfers across DMA queues so
                # round 0 isn't gated on a single ~512KB queue transfer
                nsl_dma = 4 if s == 0 else 1
                for t, ap_, w in ((qh, qhh_ap, SEG), (kk, khl_ap, SEG),
                                  (vv, vhl_ap, NKB * 128)):
                    step = w // nsl_dma
                    for z in range(nsl_dma):
                        lo = z * step
                        nc.sync.dma_start(
                            t[:, lo:lo + step],
                            ap_[:, s * w + lo:s * w + lo + step])
                qh_sb.append(qh)
                k_sb.append(kk)
                v_sb.append(vv)

            ot_tiles = {}
            oseg_tiles = {}
            pend1, pend2 = [], []  # PV work lagged by 1 and 2 rounds

            def flush(items):
                for p1ref, i, u in items:
                    cid, kb = divmod(u, NKB)
                    s, c = divmod(cid, 4)
                    if kb == 0:
                        ot_tiles[cid] = otp.tile([128, 512], F32, tag="ot",
                                                 name=f"ot{cid}")
                    vsl = slice(kb * 128, (kb + 1) * 128)
                    psl = slice(i * 512, (i + 1) * 512)
                    nc.tensor.matmul(ot_tiles[cid][:, :], v_sb[s][:, vsl],
                                     p1ref[:, psl],
                                     start=(kb == 0), stop=(kb == NKB - 1))
                    if kb == NKB - 1:
                        if c == 0:
                            oseg_tiles[s] = osbp.tile(
                                [128, 4 * 512], F32, tag="oseg",
                                name=f"oseg{s}")
                        osl = slice(c * 512, (c + 1) * 512)
                        nc.vector.tensor_copy(oseg_tiles[s][:, osl],
                                              ot_tiles[cid][:, :])
                        if c == 3:
                            nc.sync.dma_start(
                                out_ap[:, s * 2048:(s + 1) * 2048],
                                oseg_tiles[s][:, :])

            for r in range((NUNIT + RW - 1) // RW):
                units = range(r * RW, min((r + 1) * RW, NUNIT))
                nu = len(units)
                score = scp.tile([128, 512 * RW], F32, tag="score",
                                 name=f"score{r}")
                for i, u in enumerate(units):
                    cid, kb = divmod(u, NKB)
                    s, c = divmod(cid, 4)
                    osl = slice(i * 512, (i + 1) * 512)
                    csl = slice(c * 512, (c + 1) * 512)
                    lhsT = k_sb[s][:, kb * 128:(kb + 1) * 128]
                    nc.tensor.matmul(score[:, osl], lhsT, qh_sb[s][:, csl],
                                     start=True, stop=True)
                nsl = slice(0, 512 * nu)
                p1 = ptp.tile([128, 512 * RW], F16, tag="p1", name=f"p1_{r}")
                nc.scalar.activation(
                    p1[:, nsl], score[:, nsl],
                    mybir.ActivationFunctionType.Exp, scale=ESC,
                    bias=bias_t[:, :])
                if r < 2:
                    # startup filler: PV work arrives only after the lag-2
                    # scores->exp pipeline; keep the PE from a long idle
                    # (HAM) with dummies aimed at an OT-pool slot.
                    fill = otp.tile([128, 512], F32, tag="ot", name=f"fill{r}")
                    for z in range(3):
                        nc.tensor.matmul(fill[:, :], wsrc[:, :], wjunk[:, :],
                                         start=(z == 0), stop=(z == 2))
                flush(pend2)
                pend2 = pend1
                pend1 = [(p1, i, u) for i, u in enumerate(units)]
            flush(pend2)
            flush(pend1)

    nc.compile()
    return nc


def _prep_core(query, key, value, core):
    b, j = divmod(core, 4)
    segs = []
    for arr in (query, key, value):
        h0 = arr[b, :, j, :].reshape(4, SEG, D)
        h1 = arr[b, :, 4 + j, :].reshape(2, 4096, D)[:, 1::2, :]
        h2 = arr[b, 2::4, 8 + j, :][None]
        segs.append(np.concatenate([h0, h1, h2], axis=0))  # [7, 2048, 64]
    qs, ks, vs = segs
    # [64, NSEG*SEG] with col = s*SEG + p
    qt = (qs * QSC).transpose(2, 0, 1).reshape(D, NSEG * SEG)
    kt = (ks * QSC).transpose(2, 0, 1).reshape(D, NSEG * SEG)
    qh = qt.astype(np.float16)
    kh = kt.astype(np.float16)
    kl = (kt - kh).astype(np.float16)
    vv = vs * QSC  # [7, 2048, 64] f32, pre-scaled
    v1h = vv.astype(np.float16)
    v1l = (vv - v1h).astype(np.float16)
    # packed stationary blocks: [vh(64) | vl(ch 0-62) | ones] per k-block
    blk = np.empty((NSEG, SEG, 128), np.float16)
    blk[:, :, 0:64] = v1h
    blk[:, :, 64:127] = v1l[:, :, 0:63]
    blk[:, :, 127] = np.float16(256.0)
    vhl = blk.reshape(NSEG, NKB, 128, 128).transpose(2, 0, 1, 3).reshape(128, -1)
    return {
        "qhh": np.ascontiguousarray(np.concatenate([qh, qh], axis=0)),
        "khl": np.ascontiguousarray(np.concatenate([kh, kl], axis=0)),
        "vhl": np.ascontiguousarray(vhl),
    }


def _unshard(results, dtype):
    full = np.zeros((B, N, H, D), dtype)
    for core in range(8):
        b, j = divmod(core, 4)
        o = results[core]["out"].astype(np.float64)
        num = o[0:64].copy()
        num[0:63] += o[64:127]
        T = num / o[127:128]  # [64, 14336]
        h0 = T[:, :4 * SEG]
        full[b, :, j, :] = (h0 / (3.0 * h0.sum(1, keepdims=True))).T
        h1 = T[:, 4 * SEG:6 * SEG]
        h1 = h1 / (3.0 * h1.sum(1, keepdims=True))
        for g in range(2):
            full[b, g * 4096 + 1:(g + 1) * 4096:2, 4 + j, :] = \
                h1[:, g * SEG:(g + 1) * SEG].T
        h2 = T[:, 6 * SEG:]
        full[b, 2::4, 8 + j, :] = (h2 / (3.0 * h2.sum(1, keepdims=True))).T
    return full


def _ensure_axon_backend():
    """The bass PJRT path needs the axon/neuron jax backend. A harness may
    pin JAX_PLATFORMS=cpu for its reference; re-select axon if so."""
    import jax
    try:
        plat = jax.devices()[0].platform
    except Exception:
        plat = ""
    if plat not in ("axon", "neuron"):
        try:
            jax.config.update("jax_platforms", "axon,cpu")
            jax.devices()
        except Exception:
            pass


def kernel(query, key, value):
    _ensure_axon_backend()
    query = np.asarray(query, np.float32)
    key = np.asarray(key, np.float32)
    value = np.asarray(value, np.float32)
    assert query.shape == (B, N, H, D)

    if "nc" not in _CACHE:
        _CACHE["nc"] = _build_nc()
    nc = _CACHE["nc"]

    in_maps = [_prep_core(query, key, value, c) for c in range(8)]
    res = run_bass_kernel_spmd(nc, in_maps, core_ids=list(range(8)))
    LAST_RESULT["exec_time_ns"] = res.exec_time_ns
    return _unshard(res.results, query.dtype)


# revision 17
# speedup vs baseline: 1.0133x; 1.0017x over previous
"""Dilated attention (LongNet-style) Trainium2 kernel — v4 (fused PV).

Problem: query/key/value (2, 8192, 12, 64) f32. Three dilation groups
(segment lengths 2048/4096/8192, dilation 1/2/4, head slices 0:4/4:8/8:12).
Each group's gather produces independent dense attention over 2048-position
dilated segments; outputs are normalized per (batch, head, channel) by the
sum over all segment positions, and divided by num_groups.

Sharding: 8 cores = 2 batches x 4 "head columns". Core c owns batch c//4 and
heads {j, 4+j, 8+j} where j = c%4 -- exactly 7 dense 2048x2048x64 attention
units per core (4 + 2 + 1 segments), perfectly balanced, with all segments of
any (batch, head) on one core so normalization needs no cross-core traffic.

Precision (validated by numpy simulation of the exact arithmetic): the
x / x.sum normalization amplifies V-path errors ~140x but score/P-path
errors only ~8-15x. So K keeps an fp16 hi/lo pair packed along the
contraction dim (khl rows 0-63 = kh, 64-127 = kl; qhh rows = qh duplicated,
so scores = (kh+kl)^T qh in ONE fp16 matmul), Q and P are single fp16, and
V keeps an fp16 hi/lo pair.

v4 trick: a matmul's cost is its N (moving columns), not M, so the PV pair
+ denominator fuse into ONE matmul by packing the stationary operand as
  lhsT = [vh(ch 0-63) | vl(ch 0-62) | ones] (128 x 128):
output rows 0-63 = p1@vh, rows 64-126 = p1@vl, row 127 = softmax denom.
The HOST adds the hi/lo halves in f64. Channel 63 loses its lo-correction
(+~3e-3 error, channel-diagonal). Per 128x512 unit the PE now does just
2 matmuls (scores + fused PV) = 1296ns/round of 3, making the single ACT
exp pass (1530ns/round) the bottleneck. Sim end-to-end: ~6.6e-3 (thr 2e-2).

Device kernel (same program on all 8 cores, different data):
  - inputs: qhh/khl [128, 14336] fp16 (as above), vhl [128, 14336] fp16
    (the packed 128x128 stationary blocks per (seg, k-block)).
  - per (chunk, k-block) unit (28 q-chunks of 512 x 16 k-blocks):
      S^T = khl_blk.T @ qhh              (PE, 1 MM, PSUM f32)
      p1 = fp16(exp(S^T*0.125/65536 + ln64))   (ACT, PSUM -> SBUF)
      O'[128, 512] += vhl_blk.T @ p1     (PE, 1 MM, f32 PSUM, accum over kb)
  - O' copied PSUM->SBUF (DVE) into a per-segment staging tile, DMA'd to
    DRAM out [128, 14336] f32 once per segment (4 chunks).
Host: num = O'[0:64] (+= O'[64:127] for ch<63), den = O'[127], T = num/den,
then the group normalization (sum over positions) and /3, scattered into
the (2, 8192, 12, 64) output. Positions not in a dilated group stay zero.
"""

import os
import sys

if "/opt/trn_rl_repo" not in sys.path:
    sys.path.insert(0, "/opt/trn_rl_repo")
if "jax" not in sys.modules:
    os.environ.setdefault("JAX_PLATFORMS", "axon")

import numpy as np

import concourse.bass as bass  # noqa: F401
import concourse.mybir as mybir
import concourse.tile as tile
from concourse import bacc
from concourse.bass_utils import run_bass_kernel_spmd

F32 = mybir.dt.float32
F16 = mybir.dt.float16

B, N, H, D = 2, 8192, 12, 64
NSEG = 7           # segments per core
SEG = 2048         # dilated segment length
NCHUNK = NSEG * 4  # 512-wide q chunks per core
NKB = 16           # 128-row k blocks per segment
NUNIT = NCHUNK * NKB
RW = 3             # k-blocks per exp round (3 PSUM banks per ACT span)
QSC = np.float32(256.0)               # fp16 pre-scale for Q/K/V splits
ESC = float(0.125 / (256.0 * 256.0))  # exp scale: 1/sqrt(64) + descale
import math
PBIAS = float(math.log(64.0))         # exp bias: P *= 64, into fp16-normal range

_CACHE = {}
LAST_RESULT = {}


def _build_nc():
    nc = bacc.Bacc("TRN2", target_bir_lowering=False, debug=False,
                   enable_asserts=False, num_devices=8)
    qhh = nc.dram_tensor("qhh", [128, NSEG * SEG], F16, kind="ExternalInput")
    khl = nc.dram_tensor("khl", [128, NSEG * SEG], F16, kind="ExternalInput")
    vhl = nc.dram_tensor("vhl", [128, NSEG * NKB * 128], F16,
                         kind="ExternalInput")
    out = nc.dram_tensor("out", [128, NCHUNK * 512], F32, kind="ExternalOutput")
    qhh_ap, khl_ap, vhl_ap, out_ap = qhh.ap(), khl.ap(), vhl.ap(), out.ap()

    with tile.TileContext(nc) as tc:
        with (
            tc.tile_pool(name="inp", bufs=1) as inp,
            tc.tile_pool(name="pt", bufs=4) as ptp,
            tc.tile_pool(name="osb", bufs=2) as osbp,
            tc.tile_pool(name="score", bufs=2, space="PSUM") as scp,
            tc.tile_pool(name="ot", bufs=2, space="PSUM") as otp,
        ):
            bias_t = inp.tile([128, 1], F32, tag="bias", name="bias_t")
            nc.vector.memset(bias_t[:, :], PBIAS)

            # Minimal warm-up: 3 dummy matmuls complete a PSUM slice fast so
            # the dummy exp (and with it the ~2.7us ACT table load) fires
            # early, overlapping the first input DMAs. The HAM clock-gate
            # opens during the first real rounds (ACT-bound pipeline absorbs
            # the short cold-PE ramp).
            wsrc = inp.tile([128, 128], F16, tag="wsrc", name="wsrc")
            wjunk = inp.tile([128, 512], F16, tag="wjunk", name="wjunk")
            nc.vector.memset(wsrc[:, :], 0.01)
            nc.vector.memset(wjunk[:, :], 0.01)
            # 9 MMs ~= 3.9us of sustained PE busy at the cold 1.2GHz rate —
            # enough to open the HAM clock-gate before round 0 (which is
            # DMA-gated until ~12us anyway, so the extra warmup is free and
            # saves the ~2.5us cold-round penalty at the pipeline head).
            warm = scp.tile([128, 512 * RW], F32, tag="score", name="warm")
            for i in range(9):
                nc.tensor.matmul(warm[:, :512], wsrc[:, :], wjunk[:, :],
                                 start=(i == 0), stop=(i == 8))
            wp = ptp.tile([128, 512 * RW], F16, tag="p1", name="warmp")
            nc.scalar.activation(
                wp[:, :512], warm[:, :512],
                mybir.ActivationFunctionType.Exp, scale=ESC, bias=bias_t[:, :])

            qh_sb, k_sb, v_sb = [], [], []
            for s in range(NSEG):
                qh = inp.tile([128, SEG], F16, tag=f"qh{s}", name=f"qh{s}")
                kk = inp.tile([128, SEG], F16, tag=f"k{s}", name=f"k{s}")
                vv = inp.tile([128, NKB * 128], F16, tag=f"v{s}", name=f"v{s}")
                # split the first segment's transfers across DMA queues so
                # round 0 isn't gated on a single ~512KB queue transfer
                nsl_dma = 4 if s == 0 else 1
                for t, ap_, w in ((qh, qhh_ap, SEG), (kk, khl_ap, SEG),
                                  (vv, vhl_ap, NKB * 128)):
                    step = w // nsl_dma
                    for z in range(nsl_dma):
                        lo = z * step
                        nc.sync.dma_start(
                            t[:, lo:lo + step],
                            ap_[:, s * w + lo:s * w + lo + step])
                qh_sb.append(qh)
                k_sb.append(kk)
                v_sb.append(vv)

            ot_tiles = {}
            oseg_tiles = {}
            pend1, pend2 = [], []  # PV work lagged by 1 and 2 rounds

            def flush(items):
                for p1ref, i, u in items:
                    cid, kb = divmod(u, NKB)
                    s, c = divmod(cid, 4)
                    if kb == 0:
                        ot_tiles[cid] = otp.tile([128, 512], F32, tag="ot",
                                                 name=f"ot{cid}")
                    vsl = slice(kb * 128, (kb + 1) * 128)
                    psl = slice(i * 512, (i + 1) * 512)
                    nc.tensor.matmul(ot_tiles[cid][:, :], v_sb[s][:, vsl],
                                     p1ref[:, psl],
                                     start=(kb == 0), stop=(kb == NKB - 1))
                    if kb == NKB - 1:
                        if c == 0:
                            oseg_tiles[s] = osbp.tile(
                                [128, 4 * 512], F32, tag="oseg",
                                name=f"oseg{s}")
                        osl = slice(c * 512, (c + 1) * 512)
                        nc.vector.tensor_copy(oseg_tiles[s][:, osl],
                                              ot_tiles[cid][:, :])
                        if c == 3:
                            nc.sync.dma_start(
                                out_ap[:, s * 2048:(s + 1) * 2048],
                                oseg_tiles[s][:, :])

            for r in range((NUNIT + RW - 1) // RW):
                units = range(r * RW, min((r + 1) * RW, NUNIT))
                nu = len(units)
                score = scp.tile([128, 512 * RW], F32, tag="score",
                                 name=f"score{r}")
                for i, u in enumerate(units):
                    cid, kb = divmod(u, NKB)
                    s, c = divmod(cid, 4)
                    osl = slice(i * 512, (i + 1) * 512)
                    csl = slice(c * 512, (c + 1) * 512)
                    lhsT = k_sb[s][:, kb * 128:(kb + 1) * 128]
                    nc.tensor.matmul(score[:, osl], lhsT, qh_sb[s][:, csl],
                                     start=True, stop=True)
                nsl = slice(0, 512 * nu)
                p1 = ptp.tile([128, 512 * RW], F16, tag="p1", name=f"p1_{r}")
                nc.scalar.activation(
                    p1[:, nsl], score[:, nsl],
                    mybir.ActivationFunctionType.Exp, scale=ESC,
                    bias=bias_t[:, :])
                if r < 2:
                    # startup filler: PV work arrives only after the lag-2
                    # scores->exp pipeline; keep the PE from a long idle
                    # (HAM) with dummies aimed at an OT-pool slot.
                    fill = otp.tile([128, 512], F32, tag="ot", name=f"fill{r}")
                    for z in range(3):
                        nc.tensor.matmul(fill[:, :], wsrc[:, :], wjunk[:, :],
                                         start=(z == 0), stop=(z == 2))
                flush(pend2)
                pend2 = pend1
                pend1 = [(p1, i, u) for i, u in enumerate(units)]
            flush(pend2)
            flush(pend1)

    nc.compile()
    return nc


def _prep_core(query, key, value, core):
    b, j = divmod(core, 4)
    segs = []
    for arr in (query, key, value):
        h0 = arr[b, :, j, :].reshape(4, SEG, D)
        h1 = arr[b, :, 4 + j, :].reshape(2, 4096, D)[:, 1::2, :]
        h2 = arr[b, 2::4, 8 + j, :][None]
        segs.append(np.concatenate([h0, h1, h2], axis=0))  # [7, 2048, 64]
    qs, ks, vs = segs
    # [64, NSEG*SEG] with col = s*SEG + p
    qt = (qs * QSC).transpose(2, 0, 1).reshape(D, NSEG * SEG)
    kt = (ks * QSC).transpose(2, 0, 1).reshape(D, NSEG * SEG)
    qh = qt.astype(np.float16)
    kh = kt.astype(np.float16)
    kl = (kt - kh).astype(np.float16)
    vv = vs * QSC  # [7, 2048, 64] f32, pre-scaled
    v1h = vv.astype(np.float16)
    v1l = (vv - v1h).astype(np.float16)
    # packed stationary blocks: [vh(64) | vl(ch 0-62) | ones] per k-block
    blk = np.empty((NSEG, SEG, 128), np.float16)
    blk[:, :, 0:64] = v1h
    blk[:, :, 64:127] = v1l[:, :, 0:63]
    blk[:, :, 127] = np.float16(256.0)
    vhl = blk.reshape(NSEG, NKB, 128, 128).transpose(2, 0, 1, 3).reshape(128, -1)
    return {
        "qhh": np.ascontiguousarray(np.concatenate([qh, qh], axis=0)),
        "khl": np.ascontiguousarray(np.concatenate([kh, kl], axis=0)),
        "vhl": np.ascontiguousarray(vhl),
    }


def _unshard(results, dtype):
    full = np.zeros((B, N, H, D), dtype)
    for core in range(8):
        b, j = divmod(core, 4)
        o = results[core]["out"].astype(np.float64)
        num = o[0:64].copy()
        num[0:63] += o[64:127]
        T = num / o[127:128]  # [64, 14336]
        h0 = T[:, :4 * SEG]
        full[b, :, j, :] = (h0 / (3.0 * h0.sum(1, keepdims=True))).T
        h1 = T[:, 4 * SEG:6 * SEG]
        h1 = h1 / (3.0 * h1.sum(1, keepdims=True))
        for g in range(2):
            full[b, g * 4096 + 1:(g + 1) * 4096:2, 4 + j, :] = \
                h1[:, g * SEG:(g + 1) * SEG].T
        h2 = T[:, 6 * SEG:]
        full[b, 2::4, 8 + j, :] = (h2 / (3.0 * h2.sum(1, keepdims=True))).T
    return full


def _ensure_axon_backend():
    """The bass PJRT path needs the axon/neuron jax backend. A harness may
    pin JAX_PLATFORMS=cpu for its reference; re-select axon if so."""
    import jax
    try:
        plat = jax.devices()[0].platform
    except Exception:
        plat = ""
    if plat not in ("axon", "neuron"):
        try:
            jax.config.update("jax_platforms", "axon,cpu")
            jax.devices()
        except Exception:
            pass


def kernel(query, key, value):
    _ensure_axon_backend()
    query = np.asarray(query, np.float32)
    key = np.asarray(key, np.float32)
    value = np.asarray(value, np.float32)
    assert query.shape == (B, N, H, D)

    if "nc" not in _CACHE:
        _CACHE["nc"] = _build_nc()
    nc = _CACHE["nc"]

    in_maps = [_prep_core(query, key, value, c) for c in range(8)]
    res = run_bass_kernel_spmd(nc, in_maps, core_ids=list(range(8)))
    LAST_RESULT["exec_time_ns"] = res.exec_time_ns
    return _unshard(res.results, query.dtype)
